# revision 17
# baseline (speedup 1.0000x reference)
"""ChebConv 3-layer GNN (N=50000, E=800000) on 8 trn2 NeuronCores.

Strategy (dst-sharded SpMM):
  * Nodes are permuted (LPT bin-packing by in-degree) into 400 tiles of 128
    nodes; each of the 8 cores owns 50 consecutive tiles (6400 nodes).
  * Edges are partitioned by destination tile and padded to C chunks of 128
    edge slots per tile.  prop(x) per tile:
       - indirect-DMA gather of the (dis*x) bf16 source rows from a
         replicated DRAM table (4 SWDGE queues -> 4 Q7 core pairs generate
         descriptors in parallel),
       - PE matmul against a host-built bf16 scatter matrix B with the
         per-edge weight (-ew) folded in, accumulated over chunks in PSUM
         -> prop result in [feat, dst] (transposed) layout,
       - one DVE multiply by the broadcast dis[dst] tile per dst tile.
  * Sym-normalization  w_hat = -dis[src]*ew*dis[dst]  is split: dis[src] is
    folded into row pre-scaling (dis*x before AllGather), -ew lives in B,
    dis[dst] is the broadcast multiply; deg/dis are computed on device from
    host-grouped per-src edge weights.
  * Chebyshev algebra folded:  Tx2 = P2 - x  with  P2 = 2*A*Tx1, so
    cheb(x) = x@(W0-W2) + Tx1@W1 + A*Tx1@(2*W2)  (host folds W0-W2, 2*W2).
  * 6 AllGathers (bf16 rows) stitch layers/props together.
"""

import os
import sys
import heapq
from dataclasses import dataclass

import numpy as np
import ml_dtypes

sys.path.insert(0, "/opt/trn_rl_repo")

import concourse.bass as bass  # noqa: E402
from concourse import bacc, mybir, tile  # noqa: E402
from concourse.masks import make_identity  # noqa: E402

P = 128
BF16 = mybir.dt.bfloat16
F32 = mybir.dt.float32
I32 = mybir.dt.int32
AX = mybir.AluOpType
AF = mybir.ActivationFunctionType

bf16 = ml_dtypes.bfloat16

NQUEUES = 4


@dataclass(frozen=True)
class Dims:
    ncores: int
    T: int        # dst tiles per core
    CLO: int      # edge chunks (of 128 slots) per tile, src in low half
    CHI: int      # edge chunks per tile, src in high half
    KOUT: int     # padded max out-degree (for on-device deg computation)
    GRP: int      # tiles per gather group (must divide T)
    Din: int = 128
    H1: int = 128
    H2: int = 128
    H3: int = 64

    @property
    def C(self):
        return self.CLO + self.CHI

    @property
    def PER(self):
        return self.T * P

    @property
    def NPAD(self):
        return self.ncores * self.PER

    @property
    def HALF(self):
        return self.NPAD // 2

    @property
    def SUB(self):
        assert self.T % 2 == 0
        return self.T // 2       # tiles per sub-shard

    @property
    def SHARD(self):
        return self.SUB * P      # rows per core per sub-shard


def _gid(tg, slot, d: Dims):
    """Packed node id for (global tile, slot).  Numbering is
    (sub-shard, core, row) so that each half of the id space is exactly
    the concatenation of one AllGather over per-core sub-shards."""
    r = tg // d.T
    tt = tg % d.T
    s = tt // d.SUB
    return s * d.HALF + r * d.SHARD + (tt % d.SUB) * P + slot


def _decode_local(gid, d: Dims):
    """gid -> (core, local row) where local rows are tile-major per core."""
    s = gid // d.HALF
    rem = gid % d.HALF
    r = rem // d.SHARD
    i = rem % d.SHARD
    return r, s * d.SHARD + i


# --------------------------------------------------------------------------
# host-side graph preprocessing (index manipulation / data layout only)
# --------------------------------------------------------------------------

def _lpt_assign(counts, n_tiles):
    """Assign nodes to tiles of exactly P nodes, balancing per-tile edge
    counts (greedy largest-first).  Returns (tile, slot) per node."""
    npad = n_tiles * P
    order = np.argsort(-counts, kind="stable")
    heap = [(0, t) for t in range(n_tiles)]
    heapq.heapify(heap)
    fill = np.zeros(n_tiles, np.int64)
    tg = np.empty(npad, np.int64)
    slot = np.empty(npad, np.int64)
    for nid in order:
        while True:
            load, t = heapq.heappop(heap)
            if fill[t] < P:
                break
        tg[nid] = t
        slot[nid] = fill[t]
        fill[t] += 1
        load += int(counts[nid])
        if fill[t] < P:
            heapq.heappush(heap, (load, t))
    return tg, slot


def _prep_host(X, edge_index, edge_weight, dims: Dims):
    """Build per-core input arrays.  Returns (list of per-core dicts, loc)."""
    d = dims
    N = X.shape[0]
    npad = d.NPAD
    n_tiles = d.ncores * d.T
    src = np.asarray(edge_index[0], np.int64)
    dst = np.asarray(edge_index[1], np.int64)
    ew = np.asarray(edge_weight, np.float32)
    E = src.shape[0]

    counts = np.bincount(dst, minlength=npad)
    tg, slot = _lpt_assign(counts, n_tiles)
    perm = _gid(tg, slot, d)

    nsrc = perm[src]

    # ---- edges grouped by (dst tile, src half) --------------------------
    half_of = (nsrc >= d.HALF).astype(np.int64)
    tile_of = tg[dst]
    key = tile_of * 2 + half_of
    order = np.argsort(key, kind="stable")
    k_sorted = key[order]
    starts = np.searchsorted(k_sorted, np.arange(n_tiles * 2))
    rank = np.arange(E, dtype=np.int64) - starts[k_sorted]
    t_sorted = k_sorted // 2
    h_sorted = k_sorted % 2
    lo_max = int((rank[h_sorted == 0] + 1).max()) if (h_sorted == 0).any() else 0
    hi_max = int((rank[h_sorted == 1] + 1).max()) if (h_sorted == 1).any() else 0
    assert lo_max <= d.CLO * P, f"lo overflow {lo_max} > {d.CLO * P}"
    assert hi_max <= d.CHI * P, f"hi overflow {hi_max} > {d.CHI * P}"

    e_src = nsrc[order]
    e_dst_in = slot[dst][order]
    e_w = ew[order]
    cc = rank // P + np.where(h_sorted == 1, d.CLO, 0)   # local chunk in tile
    pp = rank % P

    # scatter matrix with the per-edge weight (-ew) folded in, bf16
    BA = np.zeros((n_tiles, P, d.C, P), bf16)
    BA[t_sorted, pp, cc, e_dst_in] = (-e_w).astype(bf16)

    # int16 token stream per (group, half):
    #   token i = (g*C_h + c_h)*128 + p, value = src - h*HALF
    ngrp = n_tiles // d.GRP
    cols_lo = d.GRP * d.CLO * 8
    cols_hi = d.GRP * d.CHI * 8
    cols_per_grp = cols_lo + cols_hi
    idx16 = np.zeros((16, ngrp * cols_per_grp), np.int16)
    g_in_grp = t_sorted % d.GRP
    grp_of = t_sorted // d.GRP
    c_h = rank // P                                       # chunk within half
    tok = np.where(h_sorted == 0,
                   (g_in_grp * d.CLO + c_h) * P + pp,
                   (g_in_grp * d.CHI + c_h) * P + pp)
    col = (grp_of * cols_per_grp + np.where(h_sorted == 1, cols_lo, 0)
           + tok // 16)
    row = tok % 16
    idx16[row, col] = (e_src - h_sorted * d.HALF).astype(np.int16)
    idx16 = np.tile(idx16, (8, 1))                        # 8 Q7 cores

    # ---- edge weights grouped by src node (for deg) ---------------------
    order_s = np.argsort(nsrc, kind="stable")
    s_sorted = nsrc[order_s]
    starts_s = np.searchsorted(s_sorted, np.arange(npad))
    rank_s = np.arange(E, dtype=np.int64) - starts_s[s_sorted]
    kmax = int((rank_s + 1).max()) if E else 1
    assert kmax <= d.KOUT, f"out-degree overflow: {kmax} > {d.KOUT}"
    ewsA = np.zeros((P, n_tiles, d.KOUT), np.float32)
    ewsA[slot[src][order_s], tg[src][order_s], rank_s] = ew[order_s]

    # ---- node features (per-core local, tile-major) ---------------------
    loc_r = tg // d.T
    loc_j = (tg % d.T) * P + slot
    Xl = np.zeros((d.ncores, d.PER, X.shape[1]), bf16)
    Xl[loc_r[:N], loc_j[:N]] = np.asarray(X, np.float32).astype(bf16)

    grp_per_core = d.T // d.GRP
    maps = []
    for c in range(d.ncores):
        tsl = slice(c * d.T, (c + 1) * d.T)
        gsl = slice(c * grp_per_core * cols_per_grp,
                    (c + 1) * grp_per_core * cols_per_grp)
        maps.append({
            "xrows": np.ascontiguousarray(Xl[c]),
            "xT": np.ascontiguousarray(Xl[c].T),
            "eidx": np.ascontiguousarray(idx16[:, gsl]),
            "Bm": np.ascontiguousarray(
                BA[tsl].reshape(d.T, P, d.C * P)),
            "ews": np.ascontiguousarray(ewsA[:, tsl, :]).reshape(P, -1),
        })
    loc = loc_r * d.PER + loc_j     # node -> flat position in concat output
    return maps, loc, perm


def compute_dims(src, dst, ncores, T, GRP):
    """Size CLO/CHI/KOUT from the edge data (LPT assignment included)."""
    src = np.asarray(src, np.int64)
    dst = np.asarray(dst, np.int64)
    d0 = Dims(ncores=ncores, T=T, CLO=1, CHI=1, KOUT=4, GRP=GRP)
    npad = ncores * T * P
    counts = np.bincount(dst, minlength=npad)
    tg, slot = _lpt_assign(counts, ncores * T)
    perm = _gid(tg, slot, d0)
    key = tg[dst] * 2 + (perm[src] >= d0.HALF)
    kc = np.bincount(key, minlength=ncores * T * 2)
    CLO = max(1, int(np.ceil(kc[0::2].max() / P)))
    CHI = max(1, int(np.ceil(kc[1::2].max() / P)))
    kout = int(np.bincount(src, minlength=1).max())
    KOUT = max(4, int(np.ceil(kout / 4) * 4))
    return Dims(ncores=ncores, T=T, CLO=CLO, CHI=CHI, KOUT=KOUT, GRP=GRP)


def _prep_weights(inp, dims: Dims):
    """Fold Chebyshev weights; returns dict of shared (replicated) arrays."""
    f32 = np.float32
    out = {}
    for l, (pre, H) in enumerate(
            [("W1", dims.H1), ("W2", dims.H2), ("W3", dims.H3)], start=1):
        W0 = np.asarray(inp[f"{pre}_0"], f32)
        W1 = np.asarray(inp[f"{pre}_1"], f32)
        W2 = np.asarray(inp[f"{pre}_2"], f32)
        out[f"l{l}w0"] = (W0 - W2).astype(bf16)
        out[f"l{l}w1"] = W1.astype(bf16)
        out[f"l{l}w2"] = (2.0 * W2).astype(bf16)   # the 2 of P2=2*A*Tx1
        out[f"b{l}"] = np.asarray(inp[f"b{l}"], f32).reshape(H, 1)
    out["wl"] = np.asarray(inp["Wl"], f32).astype(bf16).reshape(dims.H3, 1)
    out["bl"] = np.asarray(inp["bl"], f32).reshape(1, 1)
    return out


# --------------------------------------------------------------------------
# device program
# --------------------------------------------------------------------------

def build_graph(tc, ins, out_ap, d: Dims, fake_cc=False):
    """Emit the full SPMD program. `ins` maps input names to DRAM APs.
    `fake_cc` replaces collectives with local DMAs (cost-model timing)."""
    nc = tc.nc
    T, C = d.T, d.C
    PER = d.PER
    rg = [list(range(d.ncores))]

    import contextlib
    ctx = contextlib.ExitStack()
    with ctx:
        sbR = ctx.enter_context(tc.tile_pool(name="resident", bufs=1))
        sbIO = ctx.enter_context(tc.tile_pool(name="io", bufs=2))
        sbG = ctx.enter_context(tc.tile_pool(name="gather", bufs=2))
        psA = ctx.enter_context(tc.tile_pool(name="psA", bufs=3, space="PSUM"))
        psT = ctx.enter_context(tc.tile_pool(name="psT", bufs=2, space="PSUM"))
        psH = ctx.enter_context(tc.tile_pool(name="psH", bufs=2, space="PSUM"))
        psO = ctx.enter_context(tc.tile_pool(name="psO", bufs=1, space="PSUM"))
        dram = ctx.enter_context(tc.tile_pool(name="dram", bufs=1,
                                              space="DRAM"))

        # ---- resident loads ------------------------------------------------
        grp_per_core = T // d.GRP
        cols_lo = d.GRP * d.CLO * 8
        cols_hi = d.GRP * d.CHI * 8
        cols_per_grp = cols_lo + cols_hi
        idx_sb = sbR.tile([P, grp_per_core * cols_per_grp], mybir.dt.int16)
        nc.sync.dma_start(idx_sb[:], ins["eidx"][:])

        wmats = {}
        for l, hdim in ((1, d.H1), (2, d.H2), (3, d.H3)):
            din = d.Din if l == 1 else (d.H1 if l == 2 else d.H2)
            for k in ("w0", "w1", "w2"):
                t_ = sbR.tile([din, hdim], BF16, name=f"l{l}{k}sb")
                nc.sync.dma_start(t_[:], ins[f"l{l}{k}"][:])
                wmats[f"l{l}{k}"] = t_
            b_ = sbR.tile([hdim, 1], F32, name=f"b{l}sb")
            nc.sync.dma_start(b_[:], ins[f"b{l}"][:])
            wmats[f"b{l}"] = b_
        wl_sb = sbR.tile([d.H3, 1], BF16)
        nc.sync.dma_start(wl_sb[:], ins["wl"][:])
        bl_sb = sbR.tile([1, 1], F32)
        nc.sync.dma_start(bl_sb[:], ins["bl"][:])

        identB = sbR.tile([P, P], BF16)
        make_identity(nc, identB[:])
        identF = sbR.tile([P, P], F32)
        make_identity(nc, identF[:])
        ones_row = sbR.tile([1, P], F32)
        nc.gpsimd.memset(ones_row[:], 1.0)

        # activation row buffers ([feat, node] layout), rotating roles
        bufA = sbR.tile([P, PER], BF16)   # starts as x^T
        bufB = sbR.tile([P, PER], BF16)
        bufC = sbR.tile([P, PER], BF16)
        nc.sync.dma_start(bufA[:], ins["xT"][:])

        pre_row = sbR.tile([1, PER], F32)
        out_row = sbR.tile([1, PER], F32)

        # ---- deg / dis -----------------------------------------------------
        deg = sbR.tile([P, T], F32)
        with tc.tile_pool(name="ews_pool", bufs=1) as ewsp:
            ews_sb = ewsp.tile([P, T * d.KOUT], F32)
            nc.sync.dma_start(ews_sb[:], ins["ews"][:])
            for t in range(T):
                nc.vector.reduce_sum(
                    out=deg[:, t:t + 1],
                    in_=ews_sb[:, t * d.KOUT:(t + 1) * d.KOUT],
                    axis=mybir.AxisListType.X,
                )
        dmx = sbR.tile([P, T], F32)
        nc.vector.tensor_scalar(out=dmx[:], in0=deg[:], scalar1=1e-12,
                                scalar2=None, op0=AX.max)
        rec = sbR.tile([P, T], F32)
        nc.vector.reciprocal(rec[:], dmx[:])
        rt = sbR.tile([P, T], F32)
        nc.scalar.activation(out=rt[:], in_=rec[:], func=AF.Sqrt)
        msk = sbR.tile([P, T], F32)
        nc.vector.tensor_scalar(out=msk[:], in0=deg[:], scalar1=0.0,
                                scalar2=None, op0=AX.is_gt)
        dis = sbR.tile([P, T], F32)
        nc.vector.tensor_tensor(out=dis[:], in0=rt[:], in1=msk[:], op=AX.mult)

        # broadcast tile: disB[p, t*P+j] = dis[j, t] (dst-scale down parts)
        disB = sbR.tile([P, T * P], BF16)
        for t in range(T):
            tp = psO.tile([1, P], F32, space="PSUM", name="disb_tp",
                          tag="po")
            nc.tensor.transpose(out=tp[:], in_=dis[:, t:t + 1],
                                identity=identF[:])
            drow = sbIO.tile([1, P], F32, name="drow")
            nc.vector.tensor_copy(drow[:], tp[:])
            bb = psH.tile([P, P], F32, space="PSUM", name="disb_bb",
                          tag="ph")
            nc.tensor.matmul(out=bb[:], lhsT=ones_row[:], rhs=drow[:],
                             start=True, stop=True)
            nc.vector.tensor_copy(disB[:, t * P:(t + 1) * P], bb[:])

        # ---- DRAM comm buffers --------------------------------------------
        # Each stage has two sub-shard AllGathers: sub 0 covers packed node
        # ids [0, HALF) (= every core's first SUB tiles), sub 1 the rest.
        # NOTE: gathers (SWDGE dma_gather / indirect DMA) from Shared-address
        # scratchpad return garbage / crash the exec unit on this runtime, so
        # AllGather outputs stay Local.
        ag_in = [[dram.tile([d.SHARD, P], BF16, name=f"agin{i}_{s}")
                  for s in range(2)] for i in range(6)]
        full = [[dram.tile([d.HALF, P], BF16, name=f"full{i}_{s}")
                 for s in range(2)] for i in range(6)]

        def allgather(i, s):
            if fake_cc:
                nc.sync.dma_start(full[i][s][0:d.SHARD, :], ag_in[i][s][:])
                return
            nc.gpsimd.collective_compute(
                "AllGather", AX.bypass, replica_groups=rg,
                ins=[ag_in[i][s].opt()], outs=[full[i][s].opt()])

        def stage_row(t):
            """(sub-shard, row slice in that shard) for tile t's rows."""
            s = t // d.SUB
            r0 = (t % d.SUB) * P
            return s, slice(r0, r0 + P)

        # ---- initial scaled rows:  dis * x  -> AG0 ------------------------
        for t in range(T):
            xr = sbIO.tile([P, P], BF16, name="xr")
            nc.sync.dma_start(xr[:], ins["xrows"][t * P:(t + 1) * P, :])
            st = sbIO.tile([P, P], BF16, name="st")
            nc.vector.tensor_scalar(out=st[:], in0=xr[:],
                                    scalar1=dis[:, t:t + 1], scalar2=None,
                                    op0=AX.mult)
            s, rsl = stage_row(t)
            nc.sync.dma_start(ag_in[0][s][rsl, :], st[:])
            if t == d.SUB - 1:
                allgather(0, 0)
        allgather(0, 1)

        # ---- SpMM pass helper ---------------------------------------------
        def gslot(g, c):
            """G free-dim chunk index for tile-in-group g, local chunk c."""
            if c < d.CLO:
                return g * d.CLO + c
            return d.GRP * d.CLO + g * d.CHI + (c - d.CLO)

        qctr = [0]

        def spmm(full_pair, consume):
            """for each dst tile: psum[f,dst] = sum_e (-ew) x[src_e] x B."""
            for gg in range(grp_per_core):
                G = sbG.tile([P, d.GRP * C, P], BF16, tag="G", bufs=3)
                base = gg * cols_per_grp
                # split each half-gather into pieces; spread across SWDGE
                # queues so all 4 Q7 core pairs emit descriptors in parallel
                for h, (nch, src_t, goff, ioff) in enumerate((
                        (d.GRP * d.CLO, full_pair[0], 0, base),
                        (d.GRP * d.CHI, full_pair[1], d.GRP * d.CLO,
                         base + cols_lo))):
                    NPIECE = 1
                    for piece in range(NPIECE):
                        c0 = (nch * piece) // NPIECE
                        c1 = (nch * (piece + 1)) // NPIECE
                        if c1 == c0:
                            continue
                        ni = (c1 - c0) * P
                        nc.gpsimd.dma_gather(
                            out_ap=G[:, goff + c0:goff + c1, :],
                            in_ap=src_t[:],
                            idxs_ap=idx_sb[:, ioff + c0 * 8:ioff + c1 * 8],
                            num_idxs=ni, num_idxs_reg=ni, elem_size=P,
                            single_packet=False,
                            queue_num=qctr[0] % NQUEUES)
                        qctr[0] += 1
                Bg = sbG.tile([P, d.GRP, C * P], BF16, tag="Bg", bufs=2)
                nc.sync.dma_start(
                    Bg[:],
                    ins["Bm"][gg * d.GRP:(gg + 1) * d.GRP].rearrange(
                        "g p x -> p g x"))
                for g in range(d.GRP):
                    t = gg * d.GRP + g
                    acc = psA.tile([P, P], F32, space="PSUM", name="acc")
                    for c in range(C):
                        nc.tensor.matmul(
                            out=acc[:], lhsT=G[:, gslot(g, c), :],
                            rhs=Bg[:, g, c * P:(c + 1) * P],
                            start=(c == 0), stop=(c == C - 1))
                    consume(t, acc)

        def rows_out(src_slice, t, agi):
            """transpose [feat,node] slice -> dis-scaled rows -> ag_in."""
            ps = psT.tile([P, P], BF16, space="PSUM", name="psTt")
            nc.tensor.transpose(out=ps[:], in_=src_slice,
                                identity=identB[:])
            rows = sbIO.tile([P, P], BF16, name="rows")
            nc.scalar.activation(out=rows[:], in_=ps[:], func=AF.Copy,
                                 scale=dis[:, t:t + 1])
            s, rsl = stage_row(t)
            nc.sync.dma_start(ag_in[agi][s][rsl, :], rows[:])
            if t == d.SUB - 1:
                allgather(agi, 0)
            elif t == T - 1:
                allgather(agi, 1)

        # ---- layers --------------------------------------------------------
        layer_io = [
            # (xT buffer, Tx1T buffer, HT buffer, din, hdim)
            (bufA, bufB, bufC, d.Din, d.H1),
            (bufC, bufB, bufA, d.H1, d.H2),
            (bufA, bufB, bufC, d.H2, d.H3),
        ]
        ag_idx = 0
        for l in (1, 2, 3):
            xT_cur, TxT, HT, din, hdim = layer_io[l - 1]
            last = l == 3
            full_in = full[ag_idx]

            def cb1(t, acc, TxT=TxT, agi=ag_idx + 1):
                sl = slice(t * P, (t + 1) * P)
                nc.vector.tensor_tensor(out=TxT[:, sl], in0=acc[:],
                                        in1=disB[:, sl], op=AX.mult)
                rows_out(TxT[:, sl], t, agi)

            spmm(full_in, cb1)
            full_tx = full[ag_idx + 1]

            w0, w1, w2 = (wmats[f"l{l}w0"], wmats[f"l{l}w1"],
                          wmats[f"l{l}w2"])
            bvec = wmats[f"b{l}"]

            def cb2(t, acc, xT_cur=xT_cur, TxT=TxT, HT=HT, w0=w0, w1=w1,
                    w2=w2, bvec=bvec, hdim=hdim, last=last,
                    agi=ag_idx + 2):
                sl = slice(t * P, (t + 1) * P)
                P2T = sbIO.tile([P, P], BF16, name="P2T")
                nc.vector.tensor_tensor(out=P2T[:], in0=acc[:],
                                        in1=disB[:, sl], op=AX.mult)
                ph = psH.tile([hdim, P], F32, space="PSUM", name="ph",
                              tag="ph")
                nc.tensor.matmul(out=ph[:], lhsT=w0[:], rhs=xT_cur[:, sl],
                                 start=True, stop=False)
                nc.tensor.matmul(out=ph[:], lhsT=w1[:], rhs=TxT[:, sl],
                                 start=False, stop=False)
                nc.tensor.matmul(out=ph[:], lhsT=w2[:], rhs=P2T[:],
                                 start=False, stop=True)
                # leaky relu fused on ACT: out = Lrelu(ph + b)
                nc.scalar.activation(out=HT[:hdim, sl], in_=ph[:],
                                     func=AF.Lrelu, bias=bvec[:],
                                     alpha=0.01)
                if not last:
                    rows_out(HT[:hdim, sl], t, agi)
                else:
                    po = psO.tile([1, P], F32, space="PSUM", name="po",
                                  tag="po")
                    nc.tensor.matmul(out=po[:], lhsT=wl_sb[:],
                                     rhs=HT[:d.H3, sl], start=True, stop=True)
                    nc.vector.tensor_copy(pre_row[:, sl], po[:])

            spmm(full_tx, cb2)
            ag_idx += 2

        nc.scalar.activation(out=out_row[:], in_=pre_row[:],
                             func=AF.Sigmoid, bias=bl_sb[:])
        nc.sync.dma_start(out_ap[:], out_row[:])


# --------------------------------------------------------------------------
# top-level kernel
# --------------------------------------------------------------------------

_CACHE = {}
LAST_RESULTS = None  # BassKernelResults of the most recent run (for profiling)


def _input_specs(d: Dims):
    return {
        "xrows": ([d.PER, P], BF16),
        "xT": ([P, d.PER], BF16),
        "eidx": ([P, (d.T // d.GRP) * (d.GRP * d.CLO * 8 + d.GRP * d.CHI * 8)],
                 mybir.dt.int16),
        "Bm": ([d.T, P, d.C * P], BF16),
        "ews": ([P, d.T * d.KOUT], F32),
        "l1w0": ([d.Din, d.H1], BF16), "l1w1": ([d.Din, d.H1], BF16),
        "l1w2": ([d.Din, d.H1], BF16), "b1": ([d.H1, 1], F32),
        "l2w0": ([d.H1, d.H2], BF16), "l2w1": ([d.H1, d.H2], BF16),
        "l2w2": ([d.H1, d.H2], BF16), "b2": ([d.H2, 1], F32),
        "l3w0": ([d.H2, d.H3], BF16), "l3w1": ([d.H2, d.H3], BF16),
        "l3w2": ([d.H2, d.H3], BF16), "b3": ([d.H3, 1], F32),
        "wl": ([d.H3, 1], BF16), "bl": ([1, 1], F32),
    }


def _build_program(d: Dims):
    key = d
    if key in _CACHE:
        return _CACHE[key]
    nc = bacc.Bacc("TRN2", target_bir_lowering=False, debug=False,
                   num_devices=d.ncores, num_swdge_queues=NQUEUES,
                   dynamic_dma_scratch_size=16384)
    ins = {}
    for name, (shape, dt) in _input_specs(d).items():
        ins[name] = nc.dram_tensor(name, shape, dt, kind="ExternalInput").ap()
    out_ap = nc.dram_tensor("out", [1, d.PER], F32, kind="ExternalOutput").ap()
    with tile.TileContext(nc) as tc:
        build_graph(tc, ins, out_ap, d)
    nc.compile()
    _CACHE[key] = nc
    return nc


def kernel(**inputs) -> np.ndarray:
    from concourse import bass_utils

    X = np.asarray(inputs["X"], np.float32)
    N = X.shape[0]
    ncores = 8
    T = 50
    GRP = 2
    npad = ncores * T * P
    assert npad >= N

    d = compute_dims(inputs["edge_index"][0], inputs["edge_index"][1],
                     ncores, T, GRP)

    core_maps, loc, _gids = _prep_host(X, inputs["edge_index"],
                                       inputs["edge_weight"], d)
    shared = _prep_weights(inputs, d)
    for m in core_maps:
        m.update(shared)

    nc = _build_program(d)
    trace = bool(int(os.environ.get("KERNEL_TRACE", "0")))
    res = bass_utils.run_bass_kernel_spmd(
        nc, core_maps, core_ids=list(range(ncores)), trace=trace)
    global LAST_RESULTS
    LAST_RESULTS = res
    out_full = np.concatenate(
        [np.asarray(res.results[c]["out"]).reshape(-1)
         for c in range(ncores)])
    return out_full[loc[:N]].astype(np.float32)


if __name__ == "__main__":
    pass


# revision 18
# speedup vs baseline: 1.0606x; 1.0606x over previous
"""ChebConv 3-layer GNN (N=50000, E=800000) on 8 trn2 NeuronCores.

Strategy (dst-sharded SpMM):
  * Nodes are permuted (LPT bin-packing by in-degree) into 400 tiles of 128
    nodes; each of the 8 cores owns 50 consecutive tiles (6400 nodes).
  * Edges are partitioned by destination tile and padded to C chunks of 128
    edge slots per tile.  prop(x) per tile:
       - indirect-DMA gather of the (dis*x) bf16 source rows from a
         replicated DRAM table (4 SWDGE queues -> 4 Q7 core pairs generate
         descriptors in parallel),
       - PE matmul against a host-built bf16 scatter matrix B with the
         per-edge weight (-ew) folded in, accumulated over chunks in PSUM
         -> prop result in [feat, dst] (transposed) layout,
       - one DVE multiply by the broadcast dis[dst] tile per dst tile.
  * Sym-normalization  w_hat = -dis[src]*ew*dis[dst]  is split: dis[src] is
    folded into row pre-scaling (dis*x before AllGather), -ew lives in B,
    dis[dst] is the broadcast multiply; deg/dis are computed on device from
    host-grouped per-src edge weights.
  * Chebyshev algebra folded:  Tx2 = P2 - x  with  P2 = 2*A*Tx1, so
    cheb(x) = x@(W0-W2) + Tx1@W1 + A*Tx1@(2*W2)  (host folds W0-W2, 2*W2).
  * 6 AllGathers (bf16 rows) stitch layers/props together.
"""

import os
import sys
import heapq
from dataclasses import dataclass

import numpy as np
import ml_dtypes

sys.path.insert(0, "/opt/trn_rl_repo")

import concourse.bass as bass  # noqa: E402
from concourse import bacc, mybir, tile  # noqa: E402
from concourse.masks import make_identity  # noqa: E402

P = 128
BF16 = mybir.dt.bfloat16
F32 = mybir.dt.float32
I32 = mybir.dt.int32
AX = mybir.AluOpType
AF = mybir.ActivationFunctionType

bf16 = ml_dtypes.bfloat16

NQUEUES = 4


@dataclass(frozen=True)
class Dims:
    ncores: int
    T: int        # dst tiles per core
    CLO: int      # edge chunks (of 128 slots) per tile, src in low half
    CHI: int      # edge chunks per tile, src in high half
    KOUT: int     # padded max out-degree (for on-device deg computation)
    GRP: int      # tiles per gather group (must divide T)
    Din: int = 128
    H1: int = 128
    H2: int = 128
    H3: int = 64

    @property
    def C(self):
        return self.CLO + self.CHI

    @property
    def PER(self):
        return self.T * P

    @property
    def NPAD(self):
        return self.ncores * self.PER

    @property
    def HALF(self):
        return self.NPAD // 2

    @property
    def SUB(self):
        assert self.T % 2 == 0
        return self.T // 2       # tiles per sub-shard

    @property
    def SHARD(self):
        return self.SUB * P      # rows per core per sub-shard


def _gid(tg, slot, d: Dims):
    """Packed node id for (global tile, slot).  Numbering is
    (sub-shard, core, row) so that each half of the id space is exactly
    the concatenation of one AllGather over per-core sub-shards."""
    r = tg // d.T
    tt = tg % d.T
    s = tt // d.SUB
    return s * d.HALF + r * d.SHARD + (tt % d.SUB) * P + slot


def _decode_local(gid, d: Dims):
    """gid -> (core, local row) where local rows are tile-major per core."""
    s = gid // d.HALF
    rem = gid % d.HALF
    r = rem // d.SHARD
    i = rem % d.SHARD
    return r, s * d.SHARD + i


# --------------------------------------------------------------------------
# host-side graph preprocessing (index manipulation / data layout only)
# --------------------------------------------------------------------------

def _lpt_assign(counts, n_tiles):
    """Assign nodes to tiles of exactly P nodes, balancing per-tile edge
    counts (greedy largest-first).  Returns (tile, slot) per node."""
    npad = n_tiles * P
    order = np.argsort(-counts, kind="stable")
    heap = [(0, t) for t in range(n_tiles)]
    heapq.heapify(heap)
    fill = np.zeros(n_tiles, np.int64)
    tg = np.empty(npad, np.int64)
    slot = np.empty(npad, np.int64)
    for nid in order:
        while True:
            load, t = heapq.heappop(heap)
            if fill[t] < P:
                break
        tg[nid] = t
        slot[nid] = fill[t]
        fill[t] += 1
        load += int(counts[nid])
        if fill[t] < P:
            heapq.heappush(heap, (load, t))
    return tg, slot


def _prep_host(X, edge_index, edge_weight, dims: Dims):
    """Build per-core input arrays.  Returns (list of per-core dicts, loc)."""
    d = dims
    N = X.shape[0]
    npad = d.NPAD
    n_tiles = d.ncores * d.T
    src = np.asarray(edge_index[0], np.int64)
    dst = np.asarray(edge_index[1], np.int64)
    ew = np.asarray(edge_weight, np.float32)
    E = src.shape[0]

    counts = np.bincount(dst, minlength=npad)
    tg, slot = _lpt_assign(counts, n_tiles)
    perm = _gid(tg, slot, d)

    nsrc = perm[src]

    # ---- edges grouped by (dst tile, src half) --------------------------
    half_of = (nsrc >= d.HALF).astype(np.int64)
    tile_of = tg[dst]
    key = tile_of * 2 + half_of
    order = np.argsort(key, kind="stable")
    k_sorted = key[order]
    starts = np.searchsorted(k_sorted, np.arange(n_tiles * 2))
    rank = np.arange(E, dtype=np.int64) - starts[k_sorted]
    t_sorted = k_sorted // 2
    h_sorted = k_sorted % 2
    lo_max = int((rank[h_sorted == 0] + 1).max()) if (h_sorted == 0).any() else 0
    hi_max = int((rank[h_sorted == 1] + 1).max()) if (h_sorted == 1).any() else 0
    assert lo_max <= d.CLO * P, f"lo overflow {lo_max} > {d.CLO * P}"
    assert hi_max <= d.CHI * P, f"hi overflow {hi_max} > {d.CHI * P}"

    e_src = nsrc[order]
    e_dst_in = slot[dst][order]
    e_w = ew[order]
    cc = rank // P + np.where(h_sorted == 1, d.CLO, 0)   # local chunk in tile
    pp = rank % P

    # scatter matrix with the per-edge weight (-ew) folded in, bf16
    BA = np.zeros((n_tiles, P, d.C, P), bf16)
    BA[t_sorted, pp, cc, e_dst_in] = (-e_w).astype(bf16)

    # int16 token stream per (group, half):
    #   token i = (g*C_h + c_h)*128 + p, value = src - h*HALF
    ngrp = n_tiles // d.GRP
    cols_lo = d.GRP * d.CLO * 8
    cols_hi = d.GRP * d.CHI * 8
    cols_per_grp = cols_lo + cols_hi
    idx16 = np.zeros((16, ngrp * cols_per_grp), np.int16)
    g_in_grp = t_sorted % d.GRP
    grp_of = t_sorted // d.GRP
    c_h = rank // P                                       # chunk within half
    tok = np.where(h_sorted == 0,
                   (g_in_grp * d.CLO + c_h) * P + pp,
                   (g_in_grp * d.CHI + c_h) * P + pp)
    col = (grp_of * cols_per_grp + np.where(h_sorted == 1, cols_lo, 0)
           + tok // 16)
    row = tok % 16
    idx16[row, col] = (e_src - h_sorted * d.HALF).astype(np.int16)
    idx16 = np.tile(idx16, (8, 1))                        # 8 Q7 cores

    # ---- edge weights grouped by src node (for deg) ---------------------
    order_s = np.argsort(nsrc, kind="stable")
    s_sorted = nsrc[order_s]
    starts_s = np.searchsorted(s_sorted, np.arange(npad))
    rank_s = np.arange(E, dtype=np.int64) - starts_s[s_sorted]
    kmax = int((rank_s + 1).max()) if E else 1
    assert kmax <= d.KOUT, f"out-degree overflow: {kmax} > {d.KOUT}"
    ewsA = np.zeros((P, n_tiles, d.KOUT), np.float32)
    ewsA[slot[src][order_s], tg[src][order_s], rank_s] = ew[order_s]

    # ---- node features (per-core local, tile-major) ---------------------
    loc_r = tg // d.T
    loc_j = (tg % d.T) * P + slot
    Xl = np.zeros((d.ncores, d.PER, X.shape[1]), bf16)
    Xl[loc_r[:N], loc_j[:N]] = np.asarray(X, np.float32).astype(bf16)

    grp_per_core = d.T // d.GRP
    maps = []
    for c in range(d.ncores):
        tsl = slice(c * d.T, (c + 1) * d.T)
        gsl = slice(c * grp_per_core * cols_per_grp,
                    (c + 1) * grp_per_core * cols_per_grp)
        maps.append({
            "xrows": np.ascontiguousarray(Xl[c]),
            "xT": np.ascontiguousarray(Xl[c].T),
            "eidx": np.ascontiguousarray(idx16[:, gsl]),
            "Bm": np.ascontiguousarray(
                BA[tsl].reshape(d.T, P, d.C * P)),
            "ews": np.ascontiguousarray(ewsA[:, tsl, :]).reshape(P, -1),
        })
    loc = loc_r * d.PER + loc_j     # node -> flat position in concat output
    return maps, loc, perm


def compute_dims(src, dst, ncores, T, GRP):
    """Size CLO/CHI/KOUT from the edge data (LPT assignment included)."""
    src = np.asarray(src, np.int64)
    dst = np.asarray(dst, np.int64)
    d0 = Dims(ncores=ncores, T=T, CLO=1, CHI=1, KOUT=4, GRP=GRP)
    npad = ncores * T * P
    counts = np.bincount(dst, minlength=npad)
    tg, slot = _lpt_assign(counts, ncores * T)
    perm = _gid(tg, slot, d0)
    key = tg[dst] * 2 + (perm[src] >= d0.HALF)
    kc = np.bincount(key, minlength=ncores * T * 2)
    CLO = max(1, int(np.ceil(kc[0::2].max() / P)))
    CHI = max(1, int(np.ceil(kc[1::2].max() / P)))
    kout = int(np.bincount(src, minlength=1).max())
    KOUT = max(4, int(np.ceil(kout / 4) * 4))
    return Dims(ncores=ncores, T=T, CLO=CLO, CHI=CHI, KOUT=KOUT, GRP=GRP)


def _prep_weights(inp, dims: Dims):
    """Fold Chebyshev weights; returns dict of shared (replicated) arrays."""
    f32 = np.float32
    out = {}
    for l, (pre, H) in enumerate(
            [("W1", dims.H1), ("W2", dims.H2), ("W3", dims.H3)], start=1):
        W0 = np.asarray(inp[f"{pre}_0"], f32)
        W1 = np.asarray(inp[f"{pre}_1"], f32)
        W2 = np.asarray(inp[f"{pre}_2"], f32)
        out[f"l{l}w0"] = (W0 - W2).astype(bf16)
        out[f"l{l}w1"] = W1.astype(bf16)
        out[f"l{l}w2"] = (2.0 * W2).astype(bf16)   # the 2 of P2=2*A*Tx1
        out[f"b{l}"] = np.asarray(inp[f"b{l}"], f32).reshape(H, 1)
    out["wl"] = np.asarray(inp["Wl"], f32).astype(bf16).reshape(dims.H3, 1)
    out["bl"] = np.asarray(inp["bl"], f32).reshape(1, 1)
    return out


# --------------------------------------------------------------------------
# device program
# --------------------------------------------------------------------------

def build_graph(tc, ins, out_ap, d: Dims, fake_cc=False):
    """Emit the full SPMD program. `ins` maps input names to DRAM APs.
    `fake_cc` replaces collectives with local DMAs (cost-model timing)."""
    nc = tc.nc
    T, C = d.T, d.C
    PER = d.PER
    rg = [list(range(d.ncores))]

    import contextlib
    ctx = contextlib.ExitStack()
    with ctx:
        sbR = ctx.enter_context(tc.tile_pool(name="resident", bufs=1))
        sbIO = ctx.enter_context(tc.tile_pool(name="io", bufs=2))
        sbG = ctx.enter_context(tc.tile_pool(name="gather", bufs=2))
        psA = ctx.enter_context(tc.tile_pool(name="psA", bufs=3, space="PSUM"))
        psT = ctx.enter_context(tc.tile_pool(name="psT", bufs=2, space="PSUM"))
        psH = ctx.enter_context(tc.tile_pool(name="psH", bufs=2, space="PSUM"))
        psO = ctx.enter_context(tc.tile_pool(name="psO", bufs=1, space="PSUM"))
        dram = ctx.enter_context(tc.tile_pool(name="dram", bufs=1,
                                              space="DRAM"))

        # ---- resident loads ------------------------------------------------
        grp_per_core = T // d.GRP
        cols_lo = d.GRP * d.CLO * 8
        cols_hi = d.GRP * d.CHI * 8
        cols_per_grp = cols_lo + cols_hi
        idx_sb = sbR.tile([P, grp_per_core * cols_per_grp], mybir.dt.int16)
        nc.sync.dma_start(idx_sb[:], ins["eidx"][:])

        wmats = {}
        for l, hdim in ((1, d.H1), (2, d.H2), (3, d.H3)):
            din = d.Din if l == 1 else (d.H1 if l == 2 else d.H2)
            for k in ("w0", "w1", "w2"):
                t_ = sbR.tile([din, hdim], BF16, name=f"l{l}{k}sb")
                nc.sync.dma_start(t_[:], ins[f"l{l}{k}"][:])
                wmats[f"l{l}{k}"] = t_
            b_ = sbR.tile([hdim, 1], F32, name=f"b{l}sb")
            nc.sync.dma_start(b_[:], ins[f"b{l}"][:])
            wmats[f"b{l}"] = b_
        wl_sb = sbR.tile([d.H3, 1], BF16)
        nc.sync.dma_start(wl_sb[:], ins["wl"][:])
        bl_sb = sbR.tile([1, 1], F32)
        nc.sync.dma_start(bl_sb[:], ins["bl"][:])

        identB = sbR.tile([P, P], BF16)
        make_identity(nc, identB[:])
        identF = sbR.tile([P, P], F32)
        make_identity(nc, identF[:])
        ones_row = sbR.tile([1, P], F32)
        nc.gpsimd.memset(ones_row[:], 1.0)

        # activation row buffers ([feat, node] layout), rotating roles
        bufA = sbR.tile([P, PER], BF16)   # starts as x^T
        bufB = sbR.tile([P, PER], BF16)
        bufC = sbR.tile([P, PER], BF16)
        nc.sync.dma_start(bufA[:], ins["xT"][:])

        pre_row = sbR.tile([1, PER], F32)
        out_row = sbR.tile([1, PER], F32)

        # ---- deg / dis -----------------------------------------------------
        deg = sbR.tile([P, T], F32)
        with tc.tile_pool(name="ews_pool", bufs=1) as ewsp:
            ews_sb = ewsp.tile([P, T * d.KOUT], F32)
            nc.sync.dma_start(ews_sb[:], ins["ews"][:])
            for t in range(T):
                nc.vector.reduce_sum(
                    out=deg[:, t:t + 1],
                    in_=ews_sb[:, t * d.KOUT:(t + 1) * d.KOUT],
                    axis=mybir.AxisListType.X,
                )
        dmx = sbR.tile([P, T], F32)
        nc.vector.tensor_scalar(out=dmx[:], in0=deg[:], scalar1=1e-12,
                                scalar2=None, op0=AX.max)
        rec = sbR.tile([P, T], F32)
        nc.vector.reciprocal(rec[:], dmx[:])
        rt = sbR.tile([P, T], F32)
        nc.scalar.activation(out=rt[:], in_=rec[:], func=AF.Sqrt)
        msk = sbR.tile([P, T], F32)
        nc.vector.tensor_scalar(out=msk[:], in0=deg[:], scalar1=0.0,
                                scalar2=None, op0=AX.is_gt)
        dis = sbR.tile([P, T], F32)
        nc.vector.tensor_tensor(out=dis[:], in0=rt[:], in1=msk[:], op=AX.mult)

        # broadcast tile: disB[p, t*P+j] = dis[j, t] (dst-scale down parts)
        disB = sbR.tile([P, T * P], BF16)
        for t in range(T):
            tp = psO.tile([1, P], F32, space="PSUM", name="disb_tp",
                          tag="po")
            nc.tensor.transpose(out=tp[:], in_=dis[:, t:t + 1],
                                identity=identF[:])
            drow = sbIO.tile([1, P], F32, name="drow")
            nc.vector.tensor_copy(drow[:], tp[:])
            bb = psH.tile([P, P], F32, space="PSUM", name="disb_bb",
                          tag="ph")
            nc.tensor.matmul(out=bb[:], lhsT=ones_row[:], rhs=drow[:],
                             start=True, stop=True)
            nc.vector.tensor_copy(disB[:, t * P:(t + 1) * P], bb[:])

        # ---- DRAM comm buffers --------------------------------------------
        # Each stage has two sub-shard AllGathers: sub 0 covers packed node
        # ids [0, HALF) (= every core's first SUB tiles), sub 1 the rest.
        # NOTE: gathers (SWDGE dma_gather / indirect DMA) from Shared-address
        # scratchpad return garbage / crash the exec unit on this runtime, so
        # AllGather outputs stay Local.
        ag_in = [[dram.tile([d.SHARD, P], BF16, name=f"agin{i}_{s}")
                  for s in range(2)] for i in range(6)]
        full = [[dram.tile([d.HALF, P], BF16, name=f"full{i}_{s}")
                 for s in range(2)] for i in range(6)]

        def allgather(i, s):
            if fake_cc:
                nc.sync.dma_start(full[i][s][0:d.SHARD, :], ag_in[i][s][:])
                return
            nc.gpsimd.collective_compute(
                "AllGather", AX.bypass, replica_groups=rg,
                ins=[ag_in[i][s].opt()], outs=[full[i][s].opt()])

        def stage_row(t):
            """(sub-shard, row slice in that shard) for tile t's rows."""
            s = t // d.SUB
            r0 = (t % d.SUB) * P
            return s, slice(r0, r0 + P)

        # ---- initial scaled rows:  dis * x  -> AG0 ------------------------
        for t in range(T):
            xr = sbIO.tile([P, P], BF16, name="xr")
            nc.sync.dma_start(xr[:], ins["xrows"][t * P:(t + 1) * P, :])
            st = sbIO.tile([P, P], BF16, name="st")
            nc.vector.tensor_scalar(out=st[:], in0=xr[:],
                                    scalar1=dis[:, t:t + 1], scalar2=None,
                                    op0=AX.mult)
            s, rsl = stage_row(t)
            nc.sync.dma_start(ag_in[0][s][rsl, :], st[:])
            if t == d.SUB - 1:
                allgather(0, 0)
        allgather(0, 1)

        # ---- SpMM pass helper ---------------------------------------------
        def gslot(g, c):
            """G free-dim chunk index for tile-in-group g, local chunk c."""
            if c < d.CLO:
                return g * d.CLO + c
            return d.GRP * d.CLO + g * d.CHI + (c - d.CLO)

        qctr = [0]

        def spmm(full_pair, consume):
            """for each dst tile: psum[f,dst] = sum_e (-ew) x[src_e] x B."""
            for gg in range(grp_per_core):
                G = sbG.tile([P, d.GRP * C, P], BF16, tag="G", bufs=2)
                base = gg * cols_per_grp
                # split each half-gather into pieces; spread across SWDGE
                # queues so all 4 Q7 core pairs emit descriptors in parallel
                for h, (nch, src_t, goff, ioff) in enumerate((
                        (d.GRP * d.CLO, full_pair[0], 0, base),
                        (d.GRP * d.CHI, full_pair[1], d.GRP * d.CLO,
                         base + cols_lo))):
                    NPIECE = 2
                    for piece in range(NPIECE):
                        c0 = (nch * piece) // NPIECE
                        c1 = (nch * (piece + 1)) // NPIECE
                        if c1 == c0:
                            continue
                        ni = (c1 - c0) * P
                        nc.gpsimd.dma_gather(
                            out_ap=G[:, goff + c0:goff + c1, :],
                            in_ap=src_t[:],
                            idxs_ap=idx_sb[:, ioff + c0 * 8:ioff + c1 * 8],
                            num_idxs=ni, num_idxs_reg=ni, elem_size=P,
                            single_packet=False,
                            queue_num=qctr[0] % NQUEUES)
                        qctr[0] += 1
                Bg = sbG.tile([P, d.GRP, C * P], BF16, tag="Bg", bufs=2)
                nc.sync.dma_start(
                    Bg[:],
                    ins["Bm"][gg * d.GRP:(gg + 1) * d.GRP].rearrange(
                        "g p x -> p g x"))
                for g in range(d.GRP):
                    t = gg * d.GRP + g
                    acc = psA.tile([P, P], F32, space="PSUM", name="acc")
                    for c in range(C):
                        nc.tensor.matmul(
                            out=acc[:], lhsT=G[:, gslot(g, c), :],
                            rhs=Bg[:, g, c * P:(c + 1) * P],
                            start=(c == 0), stop=(c == C - 1))
                    consume(t, acc)

        def rows_out(src_slice, t, agi):
            """transpose [feat,node] slice -> dis-scaled rows -> ag_in."""
            ps = psT.tile([P, P], BF16, space="PSUM", name="psTt")
            nc.tensor.transpose(out=ps[:], in_=src_slice,
                                identity=identB[:])
            rows = sbIO.tile([P, P], BF16, name="rows")
            nc.scalar.activation(out=rows[:], in_=ps[:], func=AF.Copy,
                                 scale=dis[:, t:t + 1])
            s, rsl = stage_row(t)
            nc.sync.dma_start(ag_in[agi][s][rsl, :], rows[:])
            if t == d.SUB - 1:
                allgather(agi, 0)
            elif t == T - 1:
                allgather(agi, 1)

        # ---- layers --------------------------------------------------------
        layer_io = [
            # (xT buffer, Tx1T buffer, HT buffer, din, hdim)
            (bufA, bufB, bufC, d.Din, d.H1),
            (bufC, bufB, bufA, d.H1, d.H2),
            (bufA, bufB, bufC, d.H2, d.H3),
        ]
        ag_idx = 0
        for l in (1, 2, 3):
            xT_cur, TxT, HT, din, hdim = layer_io[l - 1]
            last = l == 3
            full_in = full[ag_idx]

            def cb1(t, acc, TxT=TxT, agi=ag_idx + 1):
                sl = slice(t * P, (t + 1) * P)
                nc.vector.tensor_tensor(out=TxT[:, sl], in0=acc[:],
                                        in1=disB[:, sl], op=AX.mult)
                rows_out(TxT[:, sl], t, agi)

            spmm(full_in, cb1)
            full_tx = full[ag_idx + 1]

            w0, w1, w2 = (wmats[f"l{l}w0"], wmats[f"l{l}w1"],
                          wmats[f"l{l}w2"])
            bvec = wmats[f"b{l}"]

            def cb2(t, acc, xT_cur=xT_cur, TxT=TxT, HT=HT, w0=w0, w1=w1,
                    w2=w2, bvec=bvec, hdim=hdim, last=last,
                    agi=ag_idx + 2):
                sl = slice(t * P, (t + 1) * P)
                P2T = sbIO.tile([P, P], BF16, name="P2T")
                nc.vector.tensor_tensor(out=P2T[:], in0=acc[:],
                                        in1=disB[:, sl], op=AX.mult)
                ph = psH.tile([hdim, P], F32, space="PSUM", name="ph",
                              tag="ph")
                nc.tensor.matmul(out=ph[:], lhsT=w0[:], rhs=xT_cur[:, sl],
                                 start=True, stop=False)
                nc.tensor.matmul(out=ph[:], lhsT=w1[:], rhs=TxT[:, sl],
                                 start=False, stop=False)
                nc.tensor.matmul(out=ph[:], lhsT=w2[:], rhs=P2T[:],
                                 start=False, stop=True)
                # leaky relu fused on ACT: out = Lrelu(ph + b)
                nc.scalar.activation(out=HT[:hdim, sl], in_=ph[:],
                                     func=AF.Lrelu, bias=bvec[:],
                                     alpha=0.01)
                if not last:
                    rows_out(HT[:hdim, sl], t, agi)
                else:
                    po = psO.tile([1, P], F32, space="PSUM", name="po",
                                  tag="po")
                    nc.tensor.matmul(out=po[:], lhsT=wl_sb[:],
                                     rhs=HT[:d.H3, sl], start=True, stop=True)
                    nc.vector.tensor_copy(pre_row[:, sl], po[:])

            spmm(full_tx, cb2)
            ag_idx += 2

        nc.scalar.activation(out=out_row[:], in_=pre_row[:],
                             func=AF.Sigmoid, bias=bl_sb[:])
        nc.sync.dma_start(out_ap[:], out_row[:])


# --------------------------------------------------------------------------
# top-level kernel
# --------------------------------------------------------------------------

_CACHE = {}
LAST_RESULTS = None  # BassKernelResults of the most recent run (for profiling)


def _input_specs(d: Dims):
    return {
        "xrows": ([d.PER, P], BF16),
        "xT": ([P, d.PER], BF16),
        "eidx": ([P, (d.T // d.GRP) * (d.GRP * d.CLO * 8 + d.GRP * d.CHI * 8)],
                 mybir.dt.int16),
        "Bm": ([d.T, P, d.C * P], BF16),
        "ews": ([P, d.T * d.KOUT], F32),
        "l1w0": ([d.Din, d.H1], BF16), "l1w1": ([d.Din, d.H1], BF16),
        "l1w2": ([d.Din, d.H1], BF16), "b1": ([d.H1, 1], F32),
        "l2w0": ([d.H1, d.H2], BF16), "l2w1": ([d.H1, d.H2], BF16),
        "l2w2": ([d.H1, d.H2], BF16), "b2": ([d.H2, 1], F32),
        "l3w0": ([d.H2, d.H3], BF16), "l3w1": ([d.H2, d.H3], BF16),
        "l3w2": ([d.H2, d.H3], BF16), "b3": ([d.H3, 1], F32),
        "wl": ([d.H3, 1], BF16), "bl": ([1, 1], F32),
    }


def _build_program(d: Dims):
    key = d
    if key in _CACHE:
        return _CACHE[key]
    nc = bacc.Bacc("TRN2", target_bir_lowering=False, debug=False,
                   num_devices=d.ncores, num_swdge_queues=NQUEUES,
                   dynamic_dma_scratch_size=16384)
    ins = {}
    for name, (shape, dt) in _input_specs(d).items():
        ins[name] = nc.dram_tensor(name, shape, dt, kind="ExternalInput").ap()
    out_ap = nc.dram_tensor("out", [1, d.PER], F32, kind="ExternalOutput").ap()
    with tile.TileContext(nc) as tc:
        build_graph(tc, ins, out_ap, d)
    nc.compile()
    _CACHE[key] = nc
    return nc


def kernel(**inputs) -> np.ndarray:
    from concourse import bass_utils

    X = np.asarray(inputs["X"], np.float32)
    N = X.shape[0]
    ncores = 8
    T = 50
    GRP = 2
    npad = ncores * T * P
    assert npad >= N

    d = compute_dims(inputs["edge_index"][0], inputs["edge_index"][1],
                     ncores, T, GRP)

    core_maps, loc, _gids = _prep_host(X, inputs["edge_index"],
                                       inputs["edge_weight"], d)
    shared = _prep_weights(inputs, d)
    for m in core_maps:
        m.update(shared)

    nc = _build_program(d)
    trace = bool(int(os.environ.get("KERNEL_TRACE", "0")))
    res = bass_utils.run_bass_kernel_spmd(
        nc, core_maps, core_ids=list(range(ncores)), trace=trace)
    global LAST_RESULTS
    LAST_RESULTS = res
    out_full = np.concatenate(
        [np.asarray(res.results[c]["out"]).reshape(-1)
         for c in range(ncores)])
    return out_full[loc[:N]].astype(np.float32)


if __name__ == "__main__":
    pass
